# revision 2
# baseline (speedup 1.0000x reference)
"""LSG (local-sparse-global) block attention on 8 trn2 NeuronCores.

Shape/config facts hardcoded from the problem spec:
  n=2 h=12 t=4096 d=64, BLOCK=128, SPARSE_BLOCK=128, SF=4 -> ts=1024, g=64.
Per 128-token query block b the key set is:
  global (64, padded to 128) | sparse W1 [32b-160,32b-32) | sparse W2
  [32b+64,32b+192) | local [128(b-1), 128(b+2))            -> 704 real keys.

Sharding: n*h = 24 pairs, 3 per core (data parallel, no cross-core comm).

Device-side design (per pair, 32 blocks):
 - Host ships Q^T and K^T (d-major) with an extra 65th contraction row:
   q row = 1.0, k row = 8*mask (NEG at pads). The matmul then computes
   q.k + 8*mask so exp(0.125*x) = exp(q.k/8 + mask): masking is free.
 - scoresT chunks (keys on partitions, queries free) via 6 matmuls/block
   into PSUM; one Exp activation per 2-block batch (PSUM->SBUF, bf16).
 - ctx matmuls: stationary = values chunk with a ones column appended
   (128, 65); moving = probsT chunk. Accumulates ctx^T (64, q) AND the
   softmax denominator (row 64) in one PSUM group.
 - DVE copies (65, 256) PSUM->SBUF, DMA to HBM. Host does the final
   divide-by-denominator and the (d, t) -> (t, d) transpose (0.005% of
   the FLOPs; all matmuls/softmax run on device).
"""

import numpy as np
import ml_dtypes
from contextlib import ExitStack

import concourse.bacc as bacc
import concourse.bass as bass
import concourse.tile as tile
from concourse import mybir
from concourse.bass_utils import run_bass_kernel_spmd
from concourse.tile import add_dep_helper

N, H, T, D = 2, 12, 4096, 64
TS, G = 1024, 64
NCORES = 8
PAIRS = (N * H) // NCORES  # 3
NBLK = T // 128            # 32
NEGB = np.float32(-3.0e38)
BF16 = ml_dtypes.bfloat16

LAST_RESULTS = None  # BassKernelResults of the most recent run (for test.py)


def build_program(pairs=PAIRS):
    dt = mybir.dt
    nc = bacc.Bacc("TRN2", target_bir_lowering=False, debug=False)

    qtb = nc.dram_tensor("qtb", [pairs, 65, T], dt.bfloat16, kind="ExternalInput").ap()
    ktb = nc.dram_tensor("ktb", [pairs, 65, T + 256], dt.bfloat16, kind="ExternalInput").ap()
    stb = nc.dram_tensor("stb", [pairs, 65, TS + 320], dt.bfloat16, kind="ExternalInput").ap()
    gtb = nc.dram_tensor("gtb", [pairs, 65, 128], dt.bfloat16, kind="ExternalInput").ap()
    vlb = nc.dram_tensor("vlb", [pairs, 128, 34, D], dt.bfloat16, kind="ExternalInput").ap()
    svb = nc.dram_tensor("svb", [pairs, 128, 4, 10, D], dt.bfloat16, kind="ExternalInput").ap()
    gvb = nc.dram_tensor("gvb", [pairs, 128, D], dt.bfloat16, kind="ExternalInput").ap()
    oT = nc.dram_tensor("oT", [pairs, 65, T], dt.float32, kind="ExternalOutput").ap()

    with tile.TileContext(nc) as tc, ExitStack() as ctx:
        big = ctx.enter_context(tc.tile_pool(name="big", bufs=2))
        probs = ctx.enter_context(tc.tile_pool(name="probs", bufs=3))
        outp = ctx.enter_context(tc.tile_pool(name="outp", bufs=3))
        ps_pool = ctx.enter_context(tc.tile_pool(name="psp", bufs=2, space="PSUM"))
        cx_pool = ctx.enter_context(tc.tile_pool(name="cxp", bufs=2, space="PSUM"))

        for p in range(pairs):
            qt = big.tile([65, T], dt.bfloat16, tag="qt")
            nc.sync.dma_start(out=qt, in_=qtb[p])
            kt = big.tile([65, T + 256], dt.bfloat16, tag="kt")
            nc.sync.dma_start(out=kt, in_=ktb[p])
            st = big.tile([65, TS + 320], dt.bfloat16, tag="st")
            nc.sync.dma_start(out=st, in_=stb[p])
            gt = big.tile([65, 128], dt.bfloat16, tag="gt")
            nc.sync.dma_start(out=gt, in_=gtb[p])
            vl = big.tile([128, 34, D + 1], dt.bfloat16, tag="vl")
            nc.sync.dma_start(out=vl[:, :, 0:D], in_=vlb[p])
            nc.vector.memset(vl[:, :, D:D + 1], 1.0)
            sv = big.tile([128, 4, 10, D + 1], dt.bfloat16, tag="sv")
            nc.sync.dma_start(out=sv[:, :, :, 0:D], in_=svb[p])
            nc.vector.memset(sv[:, :, :, D:D + 1], 1.0)
            gv = big.tile([128, D + 1], dt.bfloat16, tag="gv")
            nc.sync.dma_start(out=gv[:, 0:D], in_=gvb[p])
            nc.vector.memset(gv[:, D:D + 1], 1.0)

            for i in range(NBLK // 2):
                blocks = (2 * i, 2 * i + 1)
                # --- scoresT: 6 chunks x (128 keys, 128 queries) per block
                ps = ps_pool.tile([128, 2, 768], dt.float32, tag="ps")
                for bi, b in enumerate(blocks):
                    qs = qt[:, b * 128:(b + 1) * 128]
                    for j in range(3):  # local chunks b-1, b, b+1 (padded: b+j)
                        nc.tensor.matmul(ps[:, bi, j * 128:(j + 1) * 128],
                                         kt[:, (b + j) * 128:(b + j + 1) * 128],
                                         qs, start=True, stop=True)
                    nc.tensor.matmul(ps[:, bi, 384:512],
                                     st[:, b * 32:b * 32 + 128], qs,
                                     start=True, stop=True)
                    nc.tensor.matmul(ps[:, bi, 512:640],
                                     st[:, b * 32 + 224:b * 32 + 352], qs,
                                     start=True, stop=True)
                    nc.tensor.matmul(ps[:, bi, 640:768], gt, qs,
                                     start=True, stop=True)
                # --- probsT = exp(scoresT/8): one ACT instruction per batch
                pb = probs.tile([128, 2, 768], dt.bfloat16, tag="pb")
                nc.scalar.activation(pb, ps, mybir.ActivationFunctionType.Exp,
                                     scale=0.125)
                # --- ctx^T + denominator: 12 matmuls, one PSUM group
                cx = cx_pool.tile([65, 2, 128], dt.float32, tag="cx")
                first_insts = {}
                last_insts = {}
                for bi, b in enumerate(blocks):
                    ops = []
                    for j in range(3):
                        ops.append((vl[:, b + j, :], pb[:, bi, j * 128:(j + 1) * 128]))
                    ops.append((sv[:, b % 4, b // 4, :], pb[:, bi, 384:512]))
                    w2 = b + 7
                    ops.append((sv[:, w2 % 4, w2 // 4, :], pb[:, bi, 512:640]))
                    ops.append((gv, pb[:, bi, 640:768]))
                    for oi, (lhsT, rhs) in enumerate(ops):
                        start = (bi == 0 and oi == 0)
                        stop = (bi == 1 and oi == len(ops) - 1)
                        inst = nc.tensor.matmul(cx[:, bi, :], lhsT, rhs,
                                                start=start, stop=stop)
                        if oi == 0:
                            first_insts[bi] = inst
                        if oi == len(ops) - 1:
                            last_insts[bi] = inst
                # keep the single accumulation group well-ordered:
                # B's first MM after A's first (start), B's last after A's last
                add_dep_helper(first_insts[1].ins, first_insts[0].ins, sync=False)
                add_dep_helper(last_insts[1].ins, last_insts[0].ins, sync=False)
                # --- evacuate PSUM and store
                ob = outp.tile([65, 2, 128], dt.float32, tag="ob")
                nc.vector.tensor_copy(ob, cx)
                nc.sync.dma_start(
                    out=oT[p][:, i * 256:(i + 1) * 256].rearrange(
                        "r (b c) -> r b c", b=2),
                    in_=ob)

    nc.compile()
    return nc


def _prep_pair(q, k, v, am, sk, sv, sm, gk, gv, gm):
    """Build the device-layout arrays for one (n, h) pair. All inputs fp32
    numpy: q/k/v (T, D); am (T,); sk/sv (TS, D); sm (TS,); gk/gv (G, D);
    gm (G,). Returns dict of bf16 arrays."""
    def mrow(mask_vals, total, lo, hi):
        row = np.full((total,), NEGB, np.float32)
        row[lo:hi] = np.maximum(8.0 * mask_vals, NEGB)
        return row

    qt = np.empty((65, T), np.float32)
    qt[:64] = q.T
    qt[64] = 1.0

    kt = np.zeros((65, T + 256), np.float32)
    kt[:64, 128:128 + T] = k.T
    kt[64] = mrow(am, T + 256, 128, 128 + T)

    stm = np.zeros((65, TS + 320), np.float32)
    stm[:64, 160:160 + TS] = sk.T
    stm[64] = mrow(sm, TS + 320, 160, 160 + TS)

    gt = np.zeros((65, 128), np.float32)
    gt[:64, :G] = gk.T
    gt[64] = mrow(gm, 128, 0, G)

    vpad = np.zeros((T + 256, D), np.float32)
    vpad[128:128 + T] = v
    vlb = vpad.reshape(34, 128, D).transpose(1, 0, 2)

    spad = np.zeros((TS + 320, D), np.float32)
    spad[160:160 + TS] = sv
    svb = np.zeros((128, 4, 10, D), np.float32)
    for r in range(4):
        nj = 10 if r < 3 else 9
        for j in range(nj):
            svb[:, r, j] = spad[32 * r + 128 * j: 32 * r + 128 * j + 128]

    gvb = np.zeros((128, D), np.float32)
    gvb[:G] = gv

    return dict(qtb=qt.astype(BF16), ktb=kt.astype(BF16), stb=stm.astype(BF16),
                gtb=gt.astype(BF16), vlb=vlb.astype(BF16), svb=svb.astype(BF16),
                gvb=gvb.astype(BF16))


def prep_inputs(inputs):
    """Full inputs -> list of per-core in_maps."""
    q = np.asarray(inputs["query_layer"], np.float32)
    k = np.asarray(inputs["key_layer"], np.float32)
    v = np.asarray(inputs["value_layer"], np.float32)
    am = np.asarray(inputs["attention_mask"], np.float32)[:, 0, 0, :]
    sk = np.asarray(inputs["sparse_key"], np.float32)
    sv = np.asarray(inputs["sparse_value"], np.float32)
    sm = np.asarray(inputs["sparse_mask"], np.float32)[:, 0, 0, :]
    gk = np.asarray(inputs["global_key"], np.float32)
    gv = np.asarray(inputs["global_value"], np.float32)
    gm = np.asarray(inputs["global_mask"], np.float32)[:, 0, 0, :]

    in_maps = []
    for c in range(NCORES):
        per_key = {}
        for pp in range(PAIRS):
            pair = c * PAIRS + pp
            n, h = divmod(pair, H)
            d = _prep_pair(q[n, h], k[n, h], v[n, h], am[n],
                           sk[n, h], sv[n, h], sm[n], gk[n, h], gv[n, h], gm[n])
            for name, arr in d.items():
                per_key.setdefault(name, []).append(arr)
        in_maps.append({name: np.stack(arrs) for name, arrs in per_key.items()})
    return in_maps


_prog_cache = {}


def _get_program():
    if "nc" not in _prog_cache:
        _prog_cache["nc"] = build_program()
    return _prog_cache["nc"]


def kernel(**inputs):
    global LAST_RESULTS
    nc = _get_program()
    in_maps = prep_inputs(inputs)
    res = run_bass_kernel_spmd(nc, in_maps, list(range(NCORES)))
    LAST_RESULTS = res
    out = np.empty((N, H, T, D), np.float32)
    for c in range(NCORES):
        oT = res.results[c]["oT"]  # (PAIRS, 65, T)
        for pp in range(PAIRS):
            pair = c * PAIRS + pp
            n, h = divmod(pair, H)
            out[n, h] = (oT[pp, :64] / oT[pp, 64:65]).T
    return out
